# revision 9
# baseline (speedup 1.0000x reference)
import os
import sys
sys.path.insert(0, "/opt/trn_rl_repo")
import time
import functools
import numpy as np
import scipy.sparse as sp
import jax
import jax.numpy as jnp
from jax.sharding import Mesh, PartitionSpec
from jax.experimental.shard_map import shard_map

import concourse.bass as bass
import concourse.bacc as bacc
import concourse.mybir as mybir
import concourse.tile as tile
from concourse import masks
from concourse import bass2jax

# Problem constants (hardcoded per contract)
N = 20000
T = 20
D = 64
H = 64
W = 3
NCORES = 8
PER_CORE = 2500          # stocks per core (ships unpadded)
PC_PAD = 2560            # computed stocks per core (5 chunks of 512)
C = 512                  # chunk size (stocks per half-pair)
NPAIR = 3                # pairs; pair 2 has a dummy B half
dt = mybir.dt

_cache = {}
_DBG = bool(os.environ.get("KERNEL_DEBUG"))


def _dbg(msg, t0):
    if _DBG:
        print(f"[kernel] {msg}: {time.time() - t0:.3f}s", flush=True)
    return time.time()


# attention-scalar layout inside the replicated SC tile
def _ATT(w, s, t):
    return w * 420 + s * 21 + t


def _ATTB(w, s):
    return w * 420 + s * 21 + 20


def _WW(v, w):
    return 1260 + v * 4 + w


def _WWB(v):
    return 1260 + v * 4 + 3


NSC = 1536               # padded to 3*512 for the replicate matmuls


def _build_program():
    """GRU + per-week attention + weekly attention fully on device.

    x ships int8 per week in natural stock-major layout [PER_CORE, T*D];
    the tensor engine transposes 128x64 blocks into the d-major GRU layout.
    Per (w, pair): xh_A/xh_B [128, 21*C] (rows 0:64 x_t at slot t, rows
    64:128 h_{t-1} at slot t), hs [128, 21*C] packed h (A rows 0:64, B rows
    64:128).  Attention: e[s] accumulated via scalar_tensor_tensor into a
    fp16 acc tile [128, 20*C], exp in place, tree-sum for den; probs*h in
    place on hs, tree-sum for numer; emb = numer * recip(den).  Weekly
    attention over the 3 emb tiles, output weekly fp16 [5, 64, C].
    """
    nc = bacc.Bacc("TRN2", target_bir_lowering=False, debug=False,
                   num_devices=NCORES)
    SLOTS = 21 * C
    x_ins = [nc.declare_dram_parameter(f"x{w}q", [PER_CORE, T * D], dt.int8,
                                       isOutput=False) for w in range(W)]
    wl_in = nc.declare_dram_parameter("wl", [128, W * 4 * 64], dt.float16,
                                      isOutput=False)
    bl_in = nc.declare_dram_parameter("bl", [128, W * 4], dt.float32,
                                      isOutput=False)
    sc_in = nc.declare_dram_parameter("sc", [1, NSC], dt.float32,
                                      isOutput=False)
    wk_out = nc.declare_dram_parameter("wk", [5, 64, C], dt.float16,
                                       isOutput=True)
    AF = mybir.ActivationFunctionType
    OP = mybir.AluOpType

    with tile.TileContext(nc) as tc:
        with tc.tile_pool(name="wpool", bufs=1) as wpool, \
             tc.tile_pool(name="stage", bufs=1) as stp, \
             tc.tile_pool(name="xh", bufs=1) as xhp, \
             tc.tile_pool(name="hsp", bufs=1) as hsp, \
             tc.tile_pool(name="accp", bufs=1) as accp, \
             tc.tile_pool(name="gate", bufs=1) as gp, \
             tc.tile_pool(name="embp", bufs=1) as ep, \
             tc.tile_pool(name="psum", bufs=1, space="PSUM") as pp, \
             tc.tile_pool(name="ptp", bufs=2, space="PSUM") as ptp:
            wl16 = wpool.tile([128, W * 4 * 64], dt.float16)
            wl = wpool.tile([128, W * 4 * 64], dt.float32)
            bl = wpool.tile([128, W * 4], dt.float32)
            ones = wpool.tile([1, 128], dt.float32)
            idt = wpool.tile([128, 128], dt.float16)
            SC = wpool.tile([128, NSC], dt.float32)
            nc.sync.dma_start(out=wl16[:], in_=wl_in[:, :])
            nc.scalar.copy(out=wl[:], in_=wl16[:])
            nc.sync.dma_start(out=bl[:], in_=bl_in[:, :])
            nc.vector.memset(ones[:], 1.0)
            masks.make_identity(nc, idt[:])
            # replicate the [1, NSC] scalar row across all 128 partitions
            for k in range(NSC // 512):
                sc_sb = wpool.tile([1, 512], dt.float32, tag="scsb")
                nc.sync.dma_start(out=sc_sb[:],
                                  in_=sc_in[:, k * 512:(k + 1) * 512])
                rp = pp.tile([128, 512], dt.float32, tag="rep")
                nc.tensor.matmul(out=rp[:], lhsT=ones[:], rhs=sc_sb[:],
                                 start=True, stop=True)
                nc.scalar.copy(out=SC[:, k * 512:(k + 1) * 512], in_=rp[:])

            for p in range(NPAIR):
                emb = []
                e2 = ep.tile([128, 3 * C], dt.float16, tag="e2")
                for w in range(W):
                    xh_A = xhp.tile([128, SLOTS], dt.float32, tag="xha")
                    if p < 2:
                        xh_B = xhp.tile([128, SLOTS], dt.float32, tag="xhb")
                    else:
                        xh_B = None
                    hs = hsp.tile([128, SLOTS], dt.float32, tag="hs")
                    halves = [(xh_A, 2 * p)]
                    if p < 2:
                        halves.append((xh_B, 2 * p + 1))
                    # stage + transpose natural-layout x into d-major slots
                    for xh, chunk in halves:
                        xst = []
                        for j in range(4):
                            row0 = chunk * 512 + j * 128
                            nrow = min(128, PER_CORE - row0)
                            st = stp.tile([128, T * D], dt.int8,
                                          tag=f"st{j}")
                            if nrow < 128:
                                # zero pad rows at the aligned 64 boundary
                                # BEFORE the partial DMA lands real rows
                                nc.vector.memset(st[64:128, :], 0.0)
                            nc.sync.dma_start(
                                out=st[0:nrow, :],
                                in_=x_ins[w][row0:row0 + nrow, :])
                            xq = stp.tile([128, T * D], dt.float16,
                                          tag=f"xq{j}")
                            # dequantize int8 -> fp16 (scale 1/32)
                            nc.scalar.activation(out=xq[:], in_=st[:],
                                                 func=AF.Copy,
                                                 scale=1.0 / 32.0)
                            xst.append(xq)
                        for t in range(T):
                            pt = ptp.tile([128, 512], dt.float16, tag="pt")
                            for j in range(4):
                                nc.tensor.transpose(
                                    pt[0:64, j * 128:(j + 1) * 128],
                                    xst[j][:, t * 64:(t + 1) * 64],
                                    idt[:])
                            nc.scalar.copy(
                                out=xh[0:64, t * C:(t + 1) * C],
                                in_=pt[0:64, :])
                    nc.vector.memset(xh_A[64:128, 0:C], 0.0)
                    if p < 2:
                        nc.vector.memset(xh_B[64:128, 0:C], 0.0)
                    nc.vector.memset(hs[:, 0:C], 0.0)
                    # GRU recurrence
                    for t in range(T):
                        sl = slice(t * C, (t + 1) * C)
                        sl1 = slice((t + 1) * C, (t + 2) * C)
                        ps = []
                        for g in range(4):
                            pg = pp.tile([128, C], dt.float32, tag=f"g{g}")
                            lt = wl[:, (w * 4 + g) * 64:(w * 4 + g + 1) * 64]
                            nc.tensor.matmul(out=pg[0:64, :], lhsT=lt,
                                             rhs=xh_A[:, sl],
                                             start=True, stop=True)
                            if p < 2:
                                nc.tensor.matmul(out=pg[64:128, :], lhsT=lt,
                                                 rhs=xh_B[:, sl],
                                                 start=True, stop=True,
                                                 tile_position=(0, 64))
                            ps.append(pg)
                        r = gp.tile([128, C], dt.float32, tag="r")
                        z = gp.tile([128, C], dt.float32, tag="z")
                        v = gp.tile([128, C], dt.float32, tag="v")
                        wo = gp.tile([128, C], dt.float32, tag="wo")
                        c_ = gp.tile([128, C], dt.float32, tag="c")
                        s_ = gp.tile([128, C], dt.float32, tag="s")
                        t_ = gp.tile([128, C], dt.float32, tag="t")
                        nc.scalar.activation(
                            out=r[:], in_=ps[0][:], func=AF.Sigmoid,
                            bias=bl[:, (w * 4 + 0):(w * 4 + 1)])
                        nc.scalar.activation(
                            out=z[:], in_=ps[1][:], func=AF.Sigmoid,
                            bias=bl[:, (w * 4 + 1):(w * 4 + 2)])
                        nc.vector.scalar_tensor_tensor(
                            out=v[:], in0=ps[2][:],
                            scalar=bl[:, (w * 4 + 2):(w * 4 + 3)],
                            in1=r[:], op0=OP.add, op1=OP.mult)
                        nc.vector.scalar_tensor_tensor(
                            out=wo[:], in0=ps[3][:],
                            scalar=bl[:, (w * 4 + 3):(w * 4 + 4)],
                            in1=v[:], op0=OP.add, op1=OP.add)
                        nc.scalar.activation(out=c_[:], in_=wo[:],
                                             func=AF.Tanh)
                        nc.vector.tensor_sub(out=s_[:], in0=hs[:, sl],
                                             in1=c_[:])
                        nc.vector.tensor_mul(out=t_[:], in0=z[:], in1=s_[:])
                        nc.vector.tensor_add(out=hs[:, sl1], in0=c_[:],
                                             in1=t_[:])
                        if t < T - 1:
                            nc.sync.dma_start(out=xh_A[64:128, sl1],
                                              in_=hs[0:64, sl1])
                            if p < 2:
                                nc.sync.dma_start(out=xh_B[64:128, sl1],
                                                  in_=hs[64:128, sl1])
                    # per-week attention over hs slots 1..20
                    acc = accp.tile([128, T * C], dt.float16, tag="acc")
                    with nc.allow_low_precision(reason="attn exp sums ok fp16"):
                        for s in range(T):
                            eng = nc.vector
                            osl = acc[:, s * C:(s + 1) * C]
                            eng.tensor_scalar(
                                out=osl, in0=hs[:, C:2 * C],
                                scalar1=SC[:, _ATT(w, s, 0):_ATT(w, s, 0) + 1],
                                scalar2=SC[:, _ATTB(w, s):_ATTB(w, s) + 1],
                                op0=OP.mult, op1=OP.add)
                            for t in range(1, T):
                                eng.scalar_tensor_tensor(
                                    out=osl,
                                    in0=hs[:, (t + 1) * C:(t + 2) * C],
                                    scalar=SC[:, _ATT(w, s, t):_ATT(w, s, t) + 1],
                                    in1=osl, op0=OP.mult, op1=OP.add)
                        nc.scalar.activation(out=acc[:], in_=acc[:],
                                             func=AF.Exp)
                        # probs*h in place on hs (slots 1..20)
                        nc.vector.tensor_mul(out=hs[:, C:SLOTS],
                                             in0=acc[:], in1=hs[:, C:SLOTS])
                        # den tree on acc (20 slots -> slot 0)
                        nc.vector.tensor_add(out=acc[:, 0:10 * C],
                                             in0=acc[:, 0:10 * C],
                                             in1=acc[:, 10 * C:20 * C])
                        nc.vector.tensor_add(out=acc[:, 0:5 * C],
                                             in0=acc[:, 0:5 * C],
                                             in1=acc[:, 5 * C:10 * C])
                        nc.vector.tensor_add(out=acc[:, 0:2 * C],
                                             in0=acc[:, 0:2 * C],
                                             in1=acc[:, 2 * C:4 * C])
                        nc.vector.tensor_add(out=acc[:, 0:C],
                                             in0=acc[:, 0:C],
                                             in1=acc[:, C:2 * C])
                        nc.vector.tensor_add(out=acc[:, 0:C],
                                             in0=acc[:, 0:C],
                                             in1=acc[:, 4 * C:5 * C])
                        # numer tree on hs (slots 1..20 -> slot 1)
                        nc.vector.tensor_add(out=hs[:, C:11 * C],
                                             in0=hs[:, C:11 * C],
                                             in1=hs[:, 11 * C:21 * C])
                        nc.vector.tensor_add(out=hs[:, C:6 * C],
                                             in0=hs[:, C:6 * C],
                                             in1=hs[:, 6 * C:11 * C])
                        nc.vector.tensor_add(out=hs[:, C:3 * C],
                                             in0=hs[:, C:3 * C],
                                             in1=hs[:, 3 * C:5 * C])
                        nc.vector.tensor_add(out=hs[:, C:2 * C],
                                             in0=hs[:, C:2 * C],
                                             in1=hs[:, 2 * C:3 * C])
                        nc.vector.tensor_add(out=hs[:, C:2 * C],
                                             in0=hs[:, C:2 * C],
                                             in1=hs[:, 5 * C:6 * C])
                        rden = gp.tile([128, C], dt.float32, tag="td")
                        nc.vector.reciprocal(out=rden[:], in_=acc[:, 0:C])
                        embw = ep.tile([128, C], dt.float16, tag=f"emb{w}")
                        nc.vector.tensor_mul(out=embw[:], in0=hs[:, C:2 * C],
                                             in1=rden[:])
                        emb.append(embw)
                        # weekly attention partial accumulation
                        for vv in range(W):
                            esl = e2[:, vv * C:(vv + 1) * C]
                            if w == 0:
                                nc.vector.tensor_scalar(
                                    out=esl, in0=embw[:],
                                    scalar1=SC[:, _WW(vv, 0):_WW(vv, 0) + 1],
                                    scalar2=SC[:, _WWB(vv):_WWB(vv) + 1],
                                    op0=OP.mult, op1=OP.add)
                            else:
                                nc.vector.scalar_tensor_tensor(
                                    out=esl, in0=embw[:],
                                    scalar=SC[:, _WW(vv, w):_WW(vv, w) + 1],
                                    in1=esl, op0=OP.mult, op1=OP.add)
                # weekly softmax + combine
                with nc.allow_low_precision(reason="weekly out fp16"):
                    nc.scalar.activation(out=e2[:], in_=e2[:], func=AF.Exp)
                    d2 = gp.tile([128, C], dt.float32, tag="td")
                    nc.vector.tensor_add(out=d2[:], in0=e2[:, 0:C],
                                         in1=e2[:, C:2 * C])
                    nc.vector.tensor_add(out=d2[:], in0=d2[:],
                                         in1=e2[:, 2 * C:3 * C])
                    for vv in range(W):
                        nc.vector.tensor_mul(
                            out=e2[:, vv * C:(vv + 1) * C],
                            in0=e2[:, vv * C:(vv + 1) * C], in1=emb[vv][:])
                    nc.vector.tensor_add(out=e2[:, 0:C], in0=e2[:, 0:C],
                                         in1=e2[:, C:2 * C])
                    nc.vector.tensor_add(out=e2[:, 0:C], in0=e2[:, 0:C],
                                         in1=e2[:, 2 * C:3 * C])
                    rd2 = gp.tile([128, C], dt.float32, tag="td2")
                    nc.vector.reciprocal(out=rd2[:], in_=d2[:])
                    wout = ep.tile([128, C], dt.float16, tag="wout")
                    nc.vector.tensor_mul(out=wout[:], in0=e2[:, 0:C],
                                         in1=rd2[:])
                nc.sync.dma_start(out=wk_out[2 * p], in_=wout[0:64, :])
                if p < 2:
                    nc.sync.dma_start(out=wk_out[2 * p + 1],
                                      in_=wout[64:128, :])
    nc.compile()
    return nc


def _make_runner(nc):
    """Build a persistently-cached sharded jit callable for nc.

    Mirrors the axon execute path of run_bass_kernel_spmd (bass2jax
    run_bass_via_pjrt) but constructs the jitted function ONCE so warm
    calls skip retracing, XLA lowering, and the BIR re-verify that the
    per-call path pays every invocation.
    """
    bass2jax.install_neuronx_cc_hook()
    assert nc.dbg_addr is None
    partition_name = (nc.partition_id_tensor.name
                      if nc.partition_id_tensor else None)
    in_names = []
    out_names = []
    out_avals = []
    zero_outs = []
    for alloc in nc.m.functions[0].allocations:
        if not isinstance(alloc, mybir.MemoryLocationSet):
            continue
        name = alloc.memorylocations[0].name
        if alloc.kind == "ExternalInput":
            if name != partition_name:
                in_names.append(name)
        elif alloc.kind == "ExternalOutput":
            shape = tuple(alloc.tensor_shape)
            dtype = mybir.dt.np(alloc.dtype)
            out_names.append(name)
            out_avals.append(jax.core.ShapedArray(shape, dtype))
            zero_outs.append(np.zeros((NCORES * shape[0], *shape[1:]), dtype))
    n_params = len(in_names)
    all_names = list(in_names) + list(out_names)
    if partition_name is not None:
        all_names.append(partition_name)
    donate = tuple(range(n_params, n_params + len(out_names)))

    def _body(*args):
        operands = list(args)
        if partition_name is not None:
            operands.append(bass2jax.partition_id_tensor())
        outs = bass2jax._bass_exec_p.bind(
            *operands,
            out_avals=tuple(out_avals),
            in_names=tuple(all_names),
            out_names=tuple(out_names),
            lowering_input_output_aliases=(),
            sim_require_finite=True,
            sim_require_nnan=True,
            nc=nc,
        )
        return tuple(outs)

    devices = jax.devices()[:NCORES]
    mesh = Mesh(np.asarray(devices), ("core",))
    nio = n_params + len(out_names)
    sharded = jax.jit(
        shard_map(_body, mesh=mesh,
                  in_specs=(PartitionSpec("core"),) * nio,
                  out_specs=(PartitionSpec("core"),) * len(out_names),
                  check_rep=False),
        donate_argnums=donate, keep_unused=True)
    # AOT-compile so warm calls skip the python jit dispatch layers
    in_sds = [jax.ShapeDtypeStruct((NCORES * PER_CORE, T * D), np.int8)] * W
    in_sds += [jax.ShapeDtypeStruct((NCORES * 128, W * 4 * 64), np.float16),
               jax.ShapeDtypeStruct((NCORES * 128, W * 4), np.float32),
               jax.ShapeDtypeStruct((NCORES, NSC), np.float32)]
    in_sds += [jax.ShapeDtypeStruct((NCORES * a.shape[0], *a.shape[1:]),
                                    a.dtype) for a in out_avals]
    compiled = sharded.lower(*in_sds).compile()
    return compiled, in_names, out_names, zero_outs


def _make_quant():
    cpu = jax.devices("cpu")[0]

    @functools.partial(jax.jit, device=cpu)
    def quant(x0, x1, x2):
        def q(x):
            v = jnp.clip(jnp.round(x.reshape(N, T * D) * 32.0), -127, 127)
            return v.astype(jnp.int8)
        return q(x0), q(x1), q(x2)
    return quant


def _prep_weights(gru_wih, gru_whh, gru_bih, gru_bhh, att_w, att_b, ww_w, ww_b):
    wl = np.zeros((128, W * 4 * 64), np.float32)
    bl = np.zeros((128, W * 4), np.float32)
    for w in range(W):
        wih, whh = gru_wih[w], gru_whh[w]
        bih, bhh = gru_bih[w], gru_bhh[w]
        for g, (top, bot, bias) in enumerate([
                (wih[0:64], whh[0:64], bih[0:64] + bhh[0:64]),          # r
                (wih[64:128], whh[64:128], bih[64:128] + bhh[64:128]),  # z
                (np.zeros((64, 64), np.float32), whh[128:192], bhh[128:192]),
                (wih[128:192], np.zeros((64, 64), np.float32), bih[128:192]),
        ]):
            col = (w * 4 + g) * 64
            wl[0:64, col:col + 64] = top.T
            wl[64:128, col:col + 64] = bot.T
            bl[0:64, w * 4 + g] = bias
            bl[64:128, w * 4 + g] = bias
    sc = np.zeros((1, NSC), np.float32)
    for w in range(W):
        for s in range(T):
            sc[0, _ATT(w, s, 0):_ATT(w, s, 0) + T] = att_w[w, s]
            sc[0, _ATTB(w, s)] = att_b[w, s]
    for vv in range(W):
        sc[0, _WW(vv, 0):_WW(vv, 0) + W] = ww_w[vv]
        sc[0, _WWB(vv)] = ww_b[vv]
    return wl.astype(np.float16), bl, sc


def _get_runtime():
    if "sharded" not in _cache:
        nc = _build_program()
        sharded, in_names, out_names, zero_outs = _make_runner(nc)
        assert in_names == ["x0q", "x1q", "x2q", "wl", "bl", "sc"], in_names
        assert out_names == ["wk"], out_names
        _cache["nc"] = nc
        _cache["sharded"] = sharded
        _cache["outbuf"] = zero_outs[0]
        _cache["quant"] = _make_quant()
        # warm the transport: the first 2 executes pay connection /
        # buffer warm-up on the tunnel; afterwards transfers run at
        # steady state. Uses dummy int8 inputs of the real shapes.
        dx = np.zeros((N, T * D), np.int8)
        dwl = np.zeros((NCORES * 128, W * 4 * 64), np.float16)
        dbl = np.zeros((NCORES * 128, W * 4), np.float32)
        dsc = np.zeros((NCORES, NSC), np.float32)
        for _ in range(2):
            outs = sharded(dx, dx, dx, dwl, dbl, dsc, _cache["outbuf"])
            _cache["outbuf"] = outs[0]
            outs[0].block_until_ready()
    return _cache["sharded"], _cache["quant"]


def kernel(x0, x1, x2, gru_wih, gru_whh, gru_bih, gru_bhh, att_w, att_b,
           ww_w, ww_b, gat_w, gat_att_src, gat_att_dst, gat_b,
           fus_w, fus_b, reg_w, reg_b, cls_w, cls_b, edge_index):
    t0 = time.time()
    sharded, quant = _get_runtime()
    t0 = _dbg("get_runtime", t0)

    # quantize x to int8 (scale 32); dequantized on device
    q0, q1, q2 = (np.asarray(a) for a in quant(x0, x1, x2))
    t0 = _dbg("quant", t0)

    wl, bl, sc = _prep_weights(gru_wih, gru_whh, gru_bih, gru_bhh,
                               att_w, att_b, ww_w, ww_b)
    wl_g = np.tile(wl, (NCORES, 1))
    bl_g = np.tile(bl, (NCORES, 1))
    sc_g = np.tile(sc, (NCORES, 1))
    t0 = _dbg("weights", t0)

    outs = sharded(q0, q1, q2, wl_g, bl_g, sc_g, _cache["outbuf"])
    _cache["outbuf"] = outs[0]
    t0 = _dbg("dispatch", t0)

    # the upload streams in the background; the CPU is idle until results
    # land, so do all weekly-independent GAT prep here.
    loops = np.arange(N, dtype=edge_index.dtype)
    src = np.concatenate([edge_index[0], loops])
    dst = np.concatenate([edge_index[1], loops])
    # CSR structure sorted by dst; duplicates keep their own entries
    order = np.argsort(dst, kind="stable")
    srcs = src[order]
    cnt = np.bincount(dst, minlength=N)
    indptr = np.zeros(N + 1, np.int64)
    np.cumsum(cnt, out=indptr[1:])
    t0 = _dbg("edge prep", t0)

    wk = np.asarray(outs[0])                 # (8*5, 64, C) fp16
    t0 = _dbg("fetch", t0)

    weekly = wk.reshape(NCORES, 5, 64, C).transpose(0, 1, 3, 2) \
               .reshape(NCORES, PC_PAD, H)[:, :PER_CORE] \
               .reshape(N, H).astype(np.float32)
    t0 = _dbg("unshard", t0)

    # GAT on host. softmax-max subtraction is skipped: alpha is O(1) so
    # exp never overflows, and the result is mathematically identical.
    xg = weekly @ gat_w.T
    asrc = xg @ gat_att_src
    adst = xg @ gat_att_dst
    alpha = asrc[src] + adst[dst]
    alpha = np.where(alpha > 0, alpha, np.float32(0.2) * alpha)
    ex = np.exp(alpha)
    den = np.bincount(dst, weights=ex, minlength=N)
    coef = (ex / den[dst]).astype(np.float32)
    A = sp.csr_matrix((coef[order], srcs, indptr), shape=(N, N))
    cat = (A @ xg) + gat_b
    t0 = _dbg("gat", t0)

    fus = np.concatenate([weekly, cat], axis=-1) @ fus_w.T + fus_b
    fus = np.maximum(fus, 0.0)
    reg = np.ravel(fus @ reg_w.T + reg_b)
    cls = np.ravel(1.0 / (1.0 + np.exp(-(fus @ cls_w.T + cls_b))))
    _dbg("heads", t0)
    return (reg.astype(np.float32), cls.astype(np.float32))


# revision 23
# speedup vs baseline: 1.0772x; 1.0772x over previous
import os
import sys
sys.path.insert(0, "/opt/trn_rl_repo")
import time
import functools
import numpy as np
import scipy.sparse as sp
import jax
import jax.numpy as jnp
from jax.sharding import Mesh, PartitionSpec, NamedSharding
from jax.experimental.shard_map import shard_map

import concourse.bass as bass
import concourse.bacc as bacc
import concourse.mybir as mybir
import concourse.tile as tile
from concourse import masks
from concourse import bass2jax

# Problem constants (hardcoded per contract)
N = 20000
T = 20
D = 64
H = 64
W = 3
NCORES = 8
PER_CORE = 2500          # stocks per core (ships unpadded)
PC_PAD = 2560            # computed stocks per core (5 chunks of 512)
C = 512                  # chunk size (stocks per half-pair)
NPAIR = 3                # pairs; pair 2 has a dummy B half
dt = mybir.dt

_cache = {}
_DBG = bool(os.environ.get("KERNEL_DEBUG"))


def _dbg(msg, t0):
    if _DBG:
        print(f"[kernel] {msg}: {time.time() - t0:.3f}s", flush=True)
    return time.time()


# attention-scalar layout inside the replicated SC tile
def _ATT(w, s, t):
    return w * 420 + s * 21 + t


def _ATTB(w, s):
    return w * 420 + s * 21 + 20


def _WW(v, w):
    return 1260 + v * 4 + w


def _WWB(v):
    return 1260 + v * 4 + 3


NSC = 1536               # padded to 3*512 for the replicate matmuls


def _build_program():
    """GRU + per-week attention + weekly attention fully on device.

    x ships int8 per week in natural stock-major layout [PER_CORE, T*D];
    the tensor engine transposes 128x64 blocks into the d-major GRU layout.
    Per (w, pair): xh_A/xh_B [128, 21*C] (rows 0:64 x_t at slot t, rows
    64:128 h_{t-1} at slot t), hs [128, 21*C] packed h (A rows 0:64, B rows
    64:128).  Attention: e[s] accumulated via scalar_tensor_tensor into a
    fp16 acc tile [128, 20*C], exp in place, tree-sum for den; probs*h in
    place on hs, tree-sum for numer; emb = numer * recip(den).  Weekly
    attention over the 3 emb tiles, output weekly fp16 [5, 64, C].
    """
    nc = bacc.Bacc("TRN2", target_bir_lowering=False, debug=False,
                   num_devices=NCORES)
    SLOTS = 21 * C
    x0_in = nc.declare_dram_parameter("x0q", [PER_CORE, T * D], dt.int8,
                                      isOutput=False)
    x12_in = nc.declare_dram_parameter("x12q", [2 * PER_CORE, T * D],
                                       dt.int8, isOutput=False)
    # packed weights, all fp16: cols 0:768 wl, 768:780 bl, 780:792 scP
    # (scP[r, c] = sc_flat[c*128 + r])
    wb_in = nc.declare_dram_parameter("wb", [128, W * 4 * 64 + W * 4 + 12],
                                      dt.float16, isOutput=False)
    wk_out = nc.declare_dram_parameter("wk", [5, 64, C], dt.float16,
                                       isOutput=True)
    AF = mybir.ActivationFunctionType
    OP = mybir.AluOpType
    WCOL = W * 4 * 64

    with tile.TileContext(nc) as tc:
        with tc.tile_pool(name="wpool", bufs=1) as wpool, \
             tc.tile_pool(name="stage", bufs=1) as stp, \
             tc.tile_pool(name="xh", bufs=1) as xhp, \
             tc.tile_pool(name="hsp", bufs=1) as hsp, \
             tc.tile_pool(name="accp", bufs=1) as accp, \
             tc.tile_pool(name="gate", bufs=1) as gp, \
             tc.tile_pool(name="embp", bufs=1) as ep, \
             tc.tile_pool(name="psum", bufs=1, space="PSUM") as pp, \
             tc.tile_pool(name="ptp", bufs=2, space="PSUM") as ptp:
            wb = wpool.tile([128, WCOL + W * 4 + 12], dt.float16)
            wl = wpool.tile([128, W * 4 * 64], dt.float32)
            bl = wpool.tile([128, W * 4], dt.float32)
            ones = wpool.tile([1, 128], dt.float32)
            idt = wpool.tile([128, 128], dt.float16)
            SC = wpool.tile([128, NSC], dt.float32)
            nc.sync.dma_start(out=wb[:], in_=wb_in[:, :])
            nc.scalar.copy(out=wl[:], in_=wb[:, 0:WCOL])
            nc.scalar.copy(out=bl[:], in_=wb[:, WCOL:WCOL + W * 4])
            nc.vector.memset(ones[:], 1.0)
            masks.make_identity(nc, idt[:])
            # unpack scP [128, 12] -> [12, 128] via PE transpose, flatten
            # to a [1, 1536] row via DMA, then replicate across all 128
            # partitions via ones-matmul
            scT_p = pp.tile([16, 128], dt.float16, tag="scT")
            nc.tensor.transpose(scT_p[0:12, :],
                                wb[:, WCOL + W * 4:WCOL + W * 4 + 12],
                                idt[:])
            scT = wpool.tile([16, 128], dt.float16)
            nc.scalar.copy(out=scT[0:12, :], in_=scT_p[0:12, :])
            sc_row16 = wpool.tile([1, NSC], dt.float16)
            nc.sync.dma_start(out=sc_row16[0:1, :], in_=scT[0:12, :])
            sc_row = wpool.tile([1, NSC], dt.float32)
            nc.scalar.copy(out=sc_row[:], in_=sc_row16[:])
            for k in range(NSC // 512):
                rp = pp.tile([128, 512], dt.float32, tag="rep")
                nc.tensor.matmul(out=rp[:], lhsT=ones[:],
                                 rhs=sc_row[:, k * 512:(k + 1) * 512],
                                 start=True, stop=True)
                nc.scalar.copy(out=SC[:, k * 512:(k + 1) * 512], in_=rp[:])

            for p in range(NPAIR):
                emb = []
                e2 = ep.tile([128, 3 * C], dt.float16, tag="e2")
                for w in range(W):
                    xh_A = xhp.tile([128, SLOTS], dt.float32, tag="xha")
                    if p < 2:
                        xh_B = xhp.tile([128, SLOTS], dt.float32, tag="xhb")
                    else:
                        xh_B = None
                    hs = hsp.tile([128, SLOTS], dt.float32, tag="hs")
                    halves = [(xh_A, 2 * p)]
                    if p < 2:
                        halves.append((xh_B, 2 * p + 1))
                    # stage + transpose natural-layout x into d-major slots
                    for xh, chunk in halves:
                        xst = []
                        for j in range(4):
                            row0 = chunk * 512 + j * 128
                            nrow = min(128, PER_CORE - row0)
                            if w == 0:
                                xin = x0_in[row0:row0 + nrow, :]
                            else:
                                xr0 = (w - 1) * PER_CORE + row0
                                xin = x12_in[xr0:xr0 + nrow, :]
                            st = stp.tile([128, T * D], dt.int8,
                                          tag=f"st{j}")
                            if nrow < 128:
                                # zero pad rows at the aligned 64 boundary
                                # BEFORE the partial DMA lands real rows
                                nc.vector.memset(st[64:128, :], 0.0)
                            nc.sync.dma_start(out=st[0:nrow, :], in_=xin)
                            xq = stp.tile([128, T * D], dt.float16,
                                          tag=f"xq{j}")
                            # dequantize int8 -> fp16 (scale 1/32)
                            nc.scalar.activation(out=xq[:], in_=st[:],
                                                 func=AF.Copy,
                                                 scale=1.0 / 32.0)
                            xst.append(xq)
                        for t in range(T):
                            pt = ptp.tile([128, 512], dt.float16, tag="pt")
                            for j in range(4):
                                nc.tensor.transpose(
                                    pt[0:64, j * 128:(j + 1) * 128],
                                    xst[j][:, t * 64:(t + 1) * 64],
                                    idt[:])
                            nc.scalar.copy(
                                out=xh[0:64, t * C:(t + 1) * C],
                                in_=pt[0:64, :])
                    nc.vector.memset(xh_A[64:128, 0:C], 0.0)
                    if p < 2:
                        nc.vector.memset(xh_B[64:128, 0:C], 0.0)
                    nc.vector.memset(hs[:, 0:C], 0.0)
                    # GRU recurrence
                    for t in range(T):
                        sl = slice(t * C, (t + 1) * C)
                        sl1 = slice((t + 1) * C, (t + 2) * C)
                        ps = []
                        for g in range(4):
                            pg = pp.tile([128, C], dt.float32, tag=f"g{g}")
                            lt = wl[:, (w * 4 + g) * 64:(w * 4 + g + 1) * 64]
                            nc.tensor.matmul(out=pg[0:64, :], lhsT=lt,
                                             rhs=xh_A[:, sl],
                                             start=True, stop=True)
                            if p < 2:
                                nc.tensor.matmul(out=pg[64:128, :], lhsT=lt,
                                                 rhs=xh_B[:, sl],
                                                 start=True, stop=True,
                                                 tile_position=(0, 64))
                            ps.append(pg)
                        r = gp.tile([128, C], dt.float32, tag="r")
                        z = gp.tile([128, C], dt.float32, tag="z")
                        v = gp.tile([128, C], dt.float32, tag="v")
                        wo = gp.tile([128, C], dt.float32, tag="wo")
                        c_ = gp.tile([128, C], dt.float32, tag="c")
                        s_ = gp.tile([128, C], dt.float32, tag="s")
                        t_ = gp.tile([128, C], dt.float32, tag="t")
                        nc.scalar.activation(
                            out=r[:], in_=ps[0][:], func=AF.Sigmoid,
                            bias=bl[:, (w * 4 + 0):(w * 4 + 1)])
                        nc.scalar.activation(
                            out=z[:], in_=ps[1][:], func=AF.Sigmoid,
                            bias=bl[:, (w * 4 + 1):(w * 4 + 2)])
                        nc.vector.scalar_tensor_tensor(
                            out=v[:], in0=ps[2][:],
                            scalar=bl[:, (w * 4 + 2):(w * 4 + 3)],
                            in1=r[:], op0=OP.add, op1=OP.mult)
                        nc.vector.scalar_tensor_tensor(
                            out=wo[:], in0=ps[3][:],
                            scalar=bl[:, (w * 4 + 3):(w * 4 + 4)],
                            in1=v[:], op0=OP.add, op1=OP.add)
                        nc.scalar.activation(out=c_[:], in_=wo[:],
                                             func=AF.Tanh)
                        nc.vector.tensor_sub(out=s_[:], in0=hs[:, sl],
                                             in1=c_[:])
                        nc.vector.tensor_mul(out=t_[:], in0=z[:], in1=s_[:])
                        nc.vector.tensor_add(out=hs[:, sl1], in0=c_[:],
                                             in1=t_[:])
                        if t < T - 1:
                            nc.sync.dma_start(out=xh_A[64:128, sl1],
                                              in_=hs[0:64, sl1])
                            if p < 2:
                                nc.sync.dma_start(out=xh_B[64:128, sl1],
                                                  in_=hs[64:128, sl1])
                    # per-week attention over hs slots 1..20
                    acc = accp.tile([128, T * C], dt.float16, tag="acc")
                    with nc.allow_low_precision(reason="attn exp sums ok fp16"):
                        for s in range(T):
                            eng = nc.vector
                            osl = acc[:, s * C:(s + 1) * C]
                            eng.tensor_scalar(
                                out=osl, in0=hs[:, C:2 * C],
                                scalar1=SC[:, _ATT(w, s, 0):_ATT(w, s, 0) + 1],
                                scalar2=SC[:, _ATTB(w, s):_ATTB(w, s) + 1],
                                op0=OP.mult, op1=OP.add)
                            for t in range(1, T):
                                eng.scalar_tensor_tensor(
                                    out=osl,
                                    in0=hs[:, (t + 1) * C:(t + 2) * C],
                                    scalar=SC[:, _ATT(w, s, t):_ATT(w, s, t) + 1],
                                    in1=osl, op0=OP.mult, op1=OP.add)
                        nc.scalar.activation(out=acc[:], in_=acc[:],
                                             func=AF.Exp)
                        # probs*h in place on hs (slots 1..20)
                        nc.vector.tensor_mul(out=hs[:, C:SLOTS],
                                             in0=acc[:], in1=hs[:, C:SLOTS])
                        # den tree on acc (20 slots -> slot 0)
                        nc.vector.tensor_add(out=acc[:, 0:10 * C],
                                             in0=acc[:, 0:10 * C],
                                             in1=acc[:, 10 * C:20 * C])
                        nc.vector.tensor_add(out=acc[:, 0:5 * C],
                                             in0=acc[:, 0:5 * C],
                                             in1=acc[:, 5 * C:10 * C])
                        nc.vector.tensor_add(out=acc[:, 0:2 * C],
                                             in0=acc[:, 0:2 * C],
                                             in1=acc[:, 2 * C:4 * C])
                        nc.vector.tensor_add(out=acc[:, 0:C],
                                             in0=acc[:, 0:C],
                                             in1=acc[:, C:2 * C])
                        nc.vector.tensor_add(out=acc[:, 0:C],
                                             in0=acc[:, 0:C],
                                             in1=acc[:, 4 * C:5 * C])
                        # numer tree on hs (slots 1..20 -> slot 1)
                        nc.vector.tensor_add(out=hs[:, C:11 * C],
                                             in0=hs[:, C:11 * C],
                                             in1=hs[:, 11 * C:21 * C])
                        nc.vector.tensor_add(out=hs[:, C:6 * C],
                                             in0=hs[:, C:6 * C],
                                             in1=hs[:, 6 * C:11 * C])
                        nc.vector.tensor_add(out=hs[:, C:3 * C],
                                             in0=hs[:, C:3 * C],
                                             in1=hs[:, 3 * C:5 * C])
                        nc.vector.tensor_add(out=hs[:, C:2 * C],
                                             in0=hs[:, C:2 * C],
                                             in1=hs[:, 2 * C:3 * C])
                        nc.vector.tensor_add(out=hs[:, C:2 * C],
                                             in0=hs[:, C:2 * C],
                                             in1=hs[:, 5 * C:6 * C])
                        rden = gp.tile([128, C], dt.float32, tag="td")
                        nc.vector.reciprocal(out=rden[:], in_=acc[:, 0:C])
                        embw = ep.tile([128, C], dt.float16, tag=f"emb{w}")
                        nc.vector.tensor_mul(out=embw[:], in0=hs[:, C:2 * C],
                                             in1=rden[:])
                        emb.append(embw)
                        # weekly attention partial accumulation
                        for vv in range(W):
                            esl = e2[:, vv * C:(vv + 1) * C]
                            if w == 0:
                                nc.vector.tensor_scalar(
                                    out=esl, in0=embw[:],
                                    scalar1=SC[:, _WW(vv, 0):_WW(vv, 0) + 1],
                                    scalar2=SC[:, _WWB(vv):_WWB(vv) + 1],
                                    op0=OP.mult, op1=OP.add)
                            else:
                                nc.vector.scalar_tensor_tensor(
                                    out=esl, in0=embw[:],
                                    scalar=SC[:, _WW(vv, w):_WW(vv, w) + 1],
                                    in1=esl, op0=OP.mult, op1=OP.add)
                # weekly softmax + combine
                with nc.allow_low_precision(reason="weekly out fp16"):
                    nc.scalar.activation(out=e2[:], in_=e2[:], func=AF.Exp)
                    d2 = gp.tile([128, C], dt.float32, tag="td")
                    nc.vector.tensor_add(out=d2[:], in0=e2[:, 0:C],
                                         in1=e2[:, C:2 * C])
                    nc.vector.tensor_add(out=d2[:], in0=d2[:],
                                         in1=e2[:, 2 * C:3 * C])
                    for vv in range(W):
                        nc.vector.tensor_mul(
                            out=e2[:, vv * C:(vv + 1) * C],
                            in0=e2[:, vv * C:(vv + 1) * C], in1=emb[vv][:])
                    nc.vector.tensor_add(out=e2[:, 0:C], in0=e2[:, 0:C],
                                         in1=e2[:, C:2 * C])
                    nc.vector.tensor_add(out=e2[:, 0:C], in0=e2[:, 0:C],
                                         in1=e2[:, 2 * C:3 * C])
                    rd2 = gp.tile([128, C], dt.float32, tag="td2")
                    nc.vector.reciprocal(out=rd2[:], in_=d2[:])
                    wout = ep.tile([128, C], dt.float16, tag="wout")
                    nc.vector.tensor_mul(out=wout[:], in0=e2[:, 0:C],
                                         in1=rd2[:])
                nc.sync.dma_start(out=wk_out[2 * p], in_=wout[0:64, :])
                if p < 2:
                    nc.sync.dma_start(out=wk_out[2 * p + 1],
                                      in_=wout[64:128, :])
    nc.compile()
    return nc


def _make_runner(nc):
    """Build a persistently-cached sharded jit callable for nc.

    Mirrors the axon execute path of run_bass_kernel_spmd (bass2jax
    run_bass_via_pjrt) but constructs the jitted function ONCE so warm
    calls skip retracing, XLA lowering, and the BIR re-verify that the
    per-call path pays every invocation.
    """
    bass2jax.install_neuronx_cc_hook()
    assert nc.dbg_addr is None
    partition_name = (nc.partition_id_tensor.name
                      if nc.partition_id_tensor else None)
    in_names = []
    out_names = []
    out_avals = []
    zero_outs = []
    for alloc in nc.m.functions[0].allocations:
        if not isinstance(alloc, mybir.MemoryLocationSet):
            continue
        name = alloc.memorylocations[0].name
        if alloc.kind == "ExternalInput":
            if name != partition_name:
                in_names.append(name)
        elif alloc.kind == "ExternalOutput":
            shape = tuple(alloc.tensor_shape)
            dtype = mybir.dt.np(alloc.dtype)
            out_names.append(name)
            out_avals.append(jax.core.ShapedArray(shape, dtype))
            zero_outs.append(np.zeros((NCORES * shape[0], *shape[1:]), dtype))
    n_params = len(in_names)
    all_names = list(in_names) + list(out_names)
    if partition_name is not None:
        all_names.append(partition_name)
    donate = tuple(range(n_params, n_params + len(out_names)))

    def _body(*args):
        operands = list(args)
        if partition_name is not None:
            operands.append(bass2jax.partition_id_tensor())
        outs = bass2jax._bass_exec_p.bind(
            *operands,
            out_avals=tuple(out_avals),
            in_names=tuple(all_names),
            out_names=tuple(out_names),
            lowering_input_output_aliases=(),
            sim_require_finite=True,
            sim_require_nnan=True,
            nc=nc,
        )
        return tuple(outs)

    devices = jax.devices()[:NCORES]
    mesh = Mesh(np.asarray(devices), ("core",))
    nio = n_params + len(out_names)
    sharded = jax.jit(
        shard_map(_body, mesh=mesh,
                  in_specs=(PartitionSpec("core"),) * nio,
                  out_specs=(PartitionSpec("core"),) * len(out_names),
                  check_rep=False),
        donate_argnums=donate, keep_unused=True)
    # AOT-compile so warm calls skip the python jit dispatch layers
    in_sds = [jax.ShapeDtypeStruct((NCORES * PER_CORE, T * D), np.int8),
              jax.ShapeDtypeStruct((NCORES * 2 * PER_CORE, T * D), np.int8),
              jax.ShapeDtypeStruct((NCORES * 128, W * 4 * 64 + W * 4 + 12),
                                   np.float16)]
    in_sds += [jax.ShapeDtypeStruct((NCORES * a.shape[0], *a.shape[1:]),
                                    a.dtype) for a in out_avals]
    compiled = sharded.lower(*in_sds).compile()
    return compiled, in_names, out_names, zero_outs


def _make_quant():
    cpu = jax.devices("cpu")[0]

    def _q(x):
        v = jnp.clip(jnp.round(x.reshape(N, T * D) * 32.0), -127, 127)
        return v.astype(jnp.int8)

    @functools.partial(jax.jit, device=cpu)
    def quant0(x):
        return _q(x)

    @functools.partial(jax.jit, device=cpu)
    def quant12(x1, x2):
        # core-major interleave: core c rows = [x1 rows, x2 rows]
        q = jnp.stack([_q(x1), _q(x2)])
        q = q.reshape(2, NCORES, PER_CORE, T * D).transpose(1, 0, 2, 3)
        return q.reshape(2 * N, T * D)
    return quant0, quant12


def _prep_weights(gru_wih, gru_whh, gru_bih, gru_bhh, att_w, att_b, ww_w, ww_b):
    wl = np.zeros((128, W * 4 * 64), np.float32)
    bl = np.zeros((128, W * 4), np.float32)
    for w in range(W):
        wih, whh = gru_wih[w], gru_whh[w]
        bih, bhh = gru_bih[w], gru_bhh[w]
        for g, (top, bot, bias) in enumerate([
                (wih[0:64], whh[0:64], bih[0:64] + bhh[0:64]),          # r
                (wih[64:128], whh[64:128], bih[64:128] + bhh[64:128]),  # z
                (np.zeros((64, 64), np.float32), whh[128:192], bhh[128:192]),
                (wih[128:192], np.zeros((64, 64), np.float32), bih[128:192]),
        ]):
            col = (w * 4 + g) * 64
            wl[0:64, col:col + 64] = top.T
            wl[64:128, col:col + 64] = bot.T
            bl[0:64, w * 4 + g] = bias
            bl[64:128, w * 4 + g] = bias
    sc = np.zeros((1, NSC), np.float32)
    for w in range(W):
        for s in range(T):
            sc[0, _ATT(w, s, 0):_ATT(w, s, 0) + T] = att_w[w, s]
            sc[0, _ATTB(w, s)] = att_b[w, s]
    for vv in range(W):
        sc[0, _WW(vv, 0):_WW(vv, 0) + W] = ww_w[vv]
        sc[0, _WWB(vv)] = ww_b[vv]
    # pack all-fp16: [wl | bl | scP] with scP[r, c] = sc_flat[c*128 + r]
    wb = np.empty((128, W * 4 * 64 + W * 4 + 12), np.float16)
    wb[:, 0:W * 4 * 64] = wl
    wb[:, W * 4 * 64:W * 4 * 64 + W * 4] = bl
    wb[:, W * 4 * 64 + W * 4:] = sc.reshape(12, 128).T
    return wb


def _get_runtime():
    if "sharded" not in _cache:
        nc = _build_program()
        sharded, in_names, out_names, zero_outs = _make_runner(nc)
        assert in_names == ["x0q", "x12q", "wb"], in_names
        assert out_names == ["wk"], out_names
        mesh = Mesh(np.asarray(jax.devices()[:NCORES]), ("core",))
        shard = NamedSharding(mesh, PartitionSpec("core"))
        _cache["nc"] = nc
        _cache["sharded"] = sharded
        _cache["shard"] = shard
        _cache["outbuf"] = zero_outs[0]
        quant0, quant12 = _make_quant()
        _cache["quant"] = (quant0, quant12)
        # compile the quant jits
        zx = np.zeros((N, T, D), np.float32)
        np.asarray(quant0(zx))
        np.asarray(quant12(zx, zx))
        # warm the transport: the first executes/puts pay connection /
        # buffer warm-up on the tunnel; afterwards transfers hand off
        # asynchronously and stream at steady state.
        dx0 = np.zeros((N, T * D), np.int8)
        dx12 = np.zeros((2 * N, T * D), np.int8)
        dwb = np.zeros((NCORES * 128, W * 4 * 64 + W * 4 + 12), np.float16)
        for _ in range(2):
            d0 = jax.device_put(dx0, shard)
            d12 = jax.device_put(dx12, shard)
            outs = sharded(d0, d12, dwb, _cache["outbuf"])
            _cache["outbuf"] = outs[0]
            outs[0].block_until_ready()
    return _cache["sharded"], _cache["quant"], _cache["shard"]


def kernel(x0, x1, x2, gru_wih, gru_whh, gru_bih, gru_bhh, att_w, att_b,
           ww_w, ww_b, gat_w, gat_att_src, gat_att_dst, gat_b,
           fus_w, fus_b, reg_w, reg_b, cls_w, cls_b, edge_index):
    t0 = time.time()
    sharded, (quant0, quant12), shard = _get_runtime()
    t0 = _dbg("get_runtime", t0)

    # weights first: starts the upload pipe while x still quantizes
    wb = _prep_weights(gru_wih, gru_whh, gru_bih, gru_bhh,
                       att_w, att_b, ww_w, ww_b)
    dwb = jax.device_put(np.tile(wb, (NCORES, 1)), shard)
    t0 = _dbg("weights", t0)

    # quantize x to int8 (scale 32), streaming to the devices as soon as
    # each piece is ready; dequantized on device
    d0 = jax.device_put(np.asarray(quant0(x0)), shard)
    d12 = jax.device_put(np.asarray(quant12(x1, x2)), shard)
    t0 = _dbg("quant+put", t0)

    outs = sharded(d0, d12, dwb, _cache["outbuf"])
    _cache["outbuf"] = outs[0]
    t0 = _dbg("dispatch", t0)

    # the upload streams in the background; the CPU is idle until results
    # land, so do all weekly-independent GAT prep here.
    loops = np.arange(N, dtype=edge_index.dtype)
    src = np.concatenate([edge_index[0], loops])
    dst = np.concatenate([edge_index[1], loops])
    # CSR structure sorted by dst; duplicates keep their own entries
    order = np.argsort(dst, kind="stable")
    srcs = src[order]
    cnt = np.bincount(dst, minlength=N)
    indptr = np.zeros(N + 1, np.int64)
    np.cumsum(cnt, out=indptr[1:])
    t0 = _dbg("edge prep", t0)

    wk = np.asarray(outs[0])                 # (8*5, 64, C) fp16
    t0 = _dbg("fetch", t0)

    weekly = wk.reshape(NCORES, 5, 64, C).transpose(0, 1, 3, 2) \
               .reshape(NCORES, PC_PAD, H)[:, :PER_CORE] \
               .reshape(N, H).astype(np.float32)
    t0 = _dbg("unshard", t0)

    # GAT on host. softmax-max subtraction is skipped: alpha is O(1) so
    # exp never overflows, and the result is mathematically identical.
    xg = weekly @ gat_w.T
    asrc = xg @ gat_att_src
    adst = xg @ gat_att_dst
    alpha = asrc[src] + adst[dst]
    alpha = np.where(alpha > 0, alpha, np.float32(0.2) * alpha)
    ex = np.exp(alpha)
    den = np.bincount(dst, weights=ex, minlength=N)
    coef = (ex / den[dst]).astype(np.float32)
    A = sp.csr_matrix((coef[order], srcs, indptr), shape=(N, N))
    cat = (A @ xg) + gat_b
    t0 = _dbg("gat", t0)

    fus = np.concatenate([weekly, cat], axis=-1) @ fus_w.T + fus_b
    fus = np.maximum(fus, 0.0)
    reg = np.ravel(fus @ reg_w.T + reg_b)
    cls = np.ravel(1.0 / (1.0 + np.exp(-(fus @ cls_w.T + cls_b))))
    _dbg("heads", t0)
    return (reg.astype(np.float32), cls.astype(np.float32))
